# revision 34
# baseline (speedup 1.0000x reference)
"""MoE block (AdaptFormer adapters, top-2 of 8 experts) on 8 TRN2 NeuronCores.

Data-parallel over the 8192 tokens (1024/core), router + expert adapter
weights replicated.  All-bf16 PE path, HW-measured 62.3us (baseline 91.8):
  - x ships as an exact bf16 hi/lo split (xh + xl == x to 2^-17),
    pre-transposed to [D, T]; xh is packed with the Wd chunk rows into
    one [128, TC+EB] DMA per chunk so phase 1 is paced by 8 issues;
    chunk 0 splits into three small DMAs (tiny wga first) so the first
    logit matmul starts ~2.5us earlier; the sigmoid ACT table preloads
    at t=0 so no ACT_TABLE_LOAD lands in the gating critical chain.
  - logits = (xh+xl) @ (wgh+wgl): stationary [wgh|wgl] [128,16] streams
    xh (phase 1) then xl (phase 1.5) into a [16, 512] PSUM group per
    block; the hi/lo fold happens after the token-major transpose as a
    free-dim add.  Err ~1e-5 vs the 3.6e-5 min top-2/3 gap.
  - experts run fully in bf16 (f32 PSUM accumulate); ADAPTER_SCALE is
    folded into Wu on the host (exact, 0.5 = 2^-1).
  - gating: per-tile PE transposes, one batched DVE pass over
    [128, 8tiles, 16] (top-2 softmax == sigmoid of the logit gap),
    gates cast bf16, transposed back, expanded across the 512-wide
    expert axis by a 0/1 block matmul (GB).
  - schedule keeps the PE dense to avoid HAM re-throttle: phase 1 =
    logits-hi + L1 k0/k1 chunk-paced; phase 1.5 = logits-lo + first
    half of k2; phase 2 = k2 tail + k3 with the transpose/gating/GB
    chain and L2 start interleaved between k3 chunk groups.
  - out tiles accumulate hg @ Wu in two 512-halves, drain via ACT+DVE
    to a bf16 [128,1024] tile, one DMA per tile; host converts to f32.
"""
import numpy as np
import ml_dtypes
from contextlib import ExitStack

import concourse.bass as bass
import concourse.tile as tile
from concourse.tile import add_dep_helper
from concourse import bacc, mybir
from concourse.bass_utils import run_bass_kernel_spmd

N_CORES = 8
B_DIM, S_DIM, D = 2, 4096, 1024
T = B_DIM * S_DIM          # 8192 tokens
TC = T // N_CORES          # 1024 tokens per core
E, BK = 8, 64              # experts, bottleneck
EB = E * BK                # 512 concatenated expert axis
P = 128
NTT = TC // P              # token tiles per core (8)
KC = D // P                # D chunks (8)
BC = EB // P               # bottleneck chunks (4)
LBLK = 512                 # token block
NLB = TC // LBLK           # 2
TPB = LBLK // P            # token tiles per block (4)
N_WARM = 2                 # PE warm-up matmuls while the first DMA lands
CG = 6                     # const-pack col groups

F32 = mybir.dt.float32
BF16 = mybir.dt.bfloat16
AL = mybir.AluOpType
ACTF = mybir.ActivationFunctionType
AX = mybir.AxisListType

_BUILD_CACHE = {}


def _bcast(small_ap, big_ap):
    a, b = bass.broadcast_tensor_aps(big_ap, small_ap)
    return b


def _build(include_bd: bool, include_bu: bool, reps: int = 1):
    key = (include_bd, include_bu, reps)
    if key in _BUILD_CACHE:
        return _BUILD_CACHE[key]

    nc = bacc.Bacc("TRN2", target_bir_lowering=False, debug=False,
                   num_devices=N_CORES)
    # per chunk c: rows 128c..128c+127 hold [xh_c | wd_c]
    xw_d = nc.dram_tensor("xw", [D, TC + EB], BF16, kind="ExternalInput").ap()
    xl_d = nc.dram_tensor("xl", [D, TC], BF16, kind="ExternalInput").ap()
    wu_d = nc.dram_tensor("wu", [EB, D], BF16, kind="ExternalInput").ap()
    wga_d = nc.dram_tensor("wga", [P, P], BF16, kind="ExternalInput").ap()
    # bf16 const pack: [:,0,:]=identb, [:,1+k,:]=eblk chunk k
    cs_d = nc.dram_tensor("cstb", [P, CG, P], BF16, kind="ExternalInput").ap()
    idf_d = nc.dram_tensor("identf", [P, P], F32, kind="ExternalInput").ap()
    if include_bd:
        bd_d = nc.dram_tensor("bd", [P, BC], F32, kind="ExternalInput").ap()
    if include_bu:
        bu_d = nc.dram_tensor("bu", [E, D], BF16, kind="ExternalInput").ap()
    out_d = nc.dram_tensor("out", [TC, D], BF16, kind="ExternalOutput").ap()

    with tile.TileContext(nc) as tc, ExitStack() as ctx:
        wpool = ctx.enter_context(tc.tile_pool(name="weights", bufs=1))
        gpool = ctx.enter_context(tc.tile_pool(name="gates", bufs=1))
        opool = ctx.enter_context(tc.tile_pool(name="osb", bufs=3))

        lt_ps_pool = ctx.enter_context(
            tc.tile_pool(name="ltps", bufs=2, space="PSUM"))
        ht_ps_pool = ctx.enter_context(
            tc.tile_pool(name="htps", bufs=6, space="PSUM"))

        def blk_cols(b):
            return bass.ts(b, LBLK)

        # ---- PE warm-up while the first chunk lands ----
        warmb = wpool.tile([P, LBLK], BF16, tag="warmb")
        nc.vector.memset(warmb[:], 0.001)
        # preload the sigmoid ACT table now (idle ACT) — avoids a 1.3us
        # ACT_TABLE_LOAD landing inside the gating critical chain later
        sig_warm = wpool.tile([P, 1], F32, tag="sigwarm")
        nc.scalar.activation(sig_warm[:], warmb[:, 0:1], ACTF.Sigmoid)
        warm_ps = ht_ps_pool.tile([P, LBLK], F32, tag="ht", name="warm")
        for i in range(N_WARM):
            nc.tensor.matmul(warm_ps[:], warmb[:, 0:P], warmb[:],
                             start=(i == 0), stop=(i == N_WARM - 1))

        # ---- DMAs (all on the Sync queue; issue order == priority) ----
        xw_sb = [wpool.tile([P, TC + EB], BF16, tag=f"xw{c}", name=f"xw{c}")
                 if c else None for c in range(KC)]
        xw0 = [wpool.tile([P, LBLK], BF16, tag=f"xw0{i}", name=f"xw0{i}")
               for i in range(3)]
        xl_sb = [wpool.tile([P, TC], BF16, tag=f"xl{c}", name=f"xl{c}")
                 for c in range(KC)]
        wu_sb = wpool.tile([P, BC, D], BF16, tag="wu")
        wga_sb = wpool.tile([P, P], BF16, tag="wga")
        cs_sb = wpool.tile([P, CG, P], BF16, tag="cstb")
        identf = wpool.tile([P, P], F32, tag="identf")

        # tiny wga first, then chunk 0 in three small pieces: the first
        # A-matmul only needs wga + the first 512 tokens of chunk 0.
        # odd chunks issue on the GpSimd queue in parallel — the Sync
        # queue's ~0.65us per issue otherwise paces early chunk arrival.
        nc.sync.dma_start(wga_sb[:], wga_d)
        for i in range(3):
            nc.sync.dma_start(xw0[i][:],
                              xw_d[bass.ts(0, P), bass.ts(i, LBLK)])
        for c in range(1, KC):
            eng = nc.gpsimd if c % 2 else nc.sync
            eng.dma_start(xw_sb[c][:], xw_d[bass.ts(c, P), :])
        nc.sync.dma_start(identf[:], idf_d)
        nc.sync.dma_start(cs_sb[:], cs_d)
        if include_bd:
            bd_sb = wpool.tile([P, BC], F32, tag="bd")
            nc.sync.dma_start(bd_sb[:], bd_d)
        if include_bu:
            bu_sb = wpool.tile([E, D], BF16, tag="bu")
            nc.sync.dma_start(bu_sb[:], bu_d)
        for c in range(KC):
            nc.sync.dma_start(xl_sb[c][:], xl_d[bass.ts(c, P), :])
        nc.sync.dma_start(wu_sb[:], wu_d.rearrange("(k p) d -> p k d", p=P))

        def xhb(c, b):       # [128, 512] token block b of chunk c
            if c == 0:
                return xw0[b][:]
            return xw_sb[c][:, blk_cols(b)]

        def wd(c, k):
            if c == 0:
                return xw0[2][:, k * P:(k + 1) * P]
            return xw_sb[c][:, TC + k * P:TC + (k + 1) * P]

        def wga(c):          # [128, 16] = [wgh_c | wgl_c]
            return wga_sb[:, 16 * c:16 * (c + 1)]

        identb = cs_sb[:, 0, :]

        def eblk(k):         # [8, 128]
            return cs_sb[0:E, 1 + k, :]

        for rep in range(reps):
            # ---- phase 1: chunk-paced logits-hi + L1 k0,k1 ----
            lt_ps = [lt_ps_pool.tile([2 * E, LBLK], F32, tag="lt",
                                     name=f"lt{b}") for b in range(NLB)]
            ht = {}
            for k in (0, 1):
                for b in range(NLB):
                    ht[(k, b)] = ht_ps_pool.tile([P, LBLK], F32, tag="ht",
                                                 name=f"ht{k}_{b}")
            for c in range(KC):
                for b in range(NLB):
                    nc.tensor.matmul(lt_ps[b][:], wga(c),
                                     xhb(c, b),
                                     start=(c == 0), stop=False)
                for k in (0, 1):
                    for b in range(NLB):
                        nc.tensor.matmul(ht[(k, b)][:], wd(c, k),
                                         xhb(c, b),
                                         start=(c == 0), stop=(c == KC - 1))

            # relu drains for k0,k1 go first on ACT (frees PSUM slots)
            rk = {}

            def drain_relu(k, b):
                rk[(k, b)] = wpool.tile([P, LBLK], BF16, tag=f"r{k}_{b}",
                                        name=f"r{k}_{b}")
                if include_bd:
                    nc.scalar.activation(rk[(k, b)][:], ht[(k, b)][:],
                                         ACTF.Relu, bias=bd_sb[:, k:k + 1])
                else:
                    nc.scalar.activation(rk[(k, b)][:], ht[(k, b)][:],
                                         ACTF.Relu)

            for k in (0, 1):
                for b in range(NLB):
                    drain_relu(k, b)

            # ---- phase 1.5: logits-lo, + L1 k2 for chunks 0..3 ----
            for b in range(NLB):
                ht[(2, b)] = ht_ps_pool.tile([P, LBLK], F32, tag="ht",
                                             name=f"ht2_{b}")
            for c in range(KC):
                for b in range(NLB):
                    nc.tensor.matmul(lt_ps[b][:], wga(c),
                                     xl_sb[c][:, blk_cols(b)],
                                     start=False, stop=(c == KC - 1))
                if c < 4:
                    for b in range(NLB):
                        nc.tensor.matmul(ht[(2, b)][:], wd(c, 2),
                                         xhb(c, b),
                                         start=(c == 0), stop=False)

            # logits PSUM -> SBUF (gates the lbT transposes)
            lt_sb = []
            for b in range(NLB):
                ls = gpool.tile([2 * E, LBLK], F32, tag=f"ls{b}")
                nc.scalar.copy(ls[:], lt_ps[b][:])
                lt_sb.append(ls)

            # ---- phase 2: k2 tail, transposes+gating, k3, g2T/GB, L2 ----
            for c in range(4, KC):
                for b in range(NLB):
                    nc.tensor.matmul(ht[(2, b)][:], wd(c, 2),
                                     xhb(c, b),
                                     start=False, stop=(c == KC - 1))

            # transpose logits to token-major [128, tile, 16], fold hi+lo.
            # all 8 transposes land in ONE PSUM bank (disjoint free
            # ranges), then a single DVE copy drains them — no per-tile
            # PE<->DVE ping-pong.
            lb_ps = lt_ps_pool.tile([P, NTT, 2 * E], F32, tag="lt",
                                    name="lbT")
            for t in range(NTT):
                b, bo = divmod(t, TPB)
                nc.tensor.transpose(lb_ps[:, t, :],
                                    lt_sb[b][:, bass.ts(bo, P)],
                                    identf[0:2 * E, 0:2 * E])
            l_t2 = gpool.tile([P, NTT, 2 * E], F32, tag="l_t2")
            nc.vector.tensor_copy(l_t2[:], lb_ps[:])
            l_t = gpool.tile([P, NTT, E], F32, tag="l_t")
            nc.vector.tensor_tensor(l_t[:], l_t2[:, :, 0:E],
                                    l_t2[:, :, E:2 * E], op=AL.add)

            for b in range(NLB):
                drain_relu(2, b)

            # ---- batched gating math on DVE (runs under k3) ----
            m1 = gpool.tile([P, NTT, 1], F32, tag="m1")
            nc.vector.tensor_reduce(m1[:, :, 0], l_t[:], AX.X, AL.max)
            mask1 = gpool.tile([P, NTT, E], F32, tag="mask1")
            nc.vector.tensor_tensor(mask1[:], l_t[:],
                                    _bcast(m1[:], l_t[:]), op=AL.is_ge)
            lm = gpool.tile([P, NTT, E], F32, tag="lm")
            nc.vector.scalar_tensor_tensor(lm[:], mask1[:], -1e30, l_t[:],
                                           op0=AL.mult, op1=AL.add)
            m2 = gpool.tile([P, NTT, 1], F32, tag="m2")
            nc.vector.tensor_reduce(m2[:, :, 0], lm[:], AX.X, AL.max)
            delta = gpool.tile([P, NTT, 1], F32, tag="delta")
            nc.vector.tensor_tensor(delta[:], m2[:], m1[:], op=AL.subtract)
            s2 = gpool.tile([P, NTT, 1], F32, tag="s2")
            nc.scalar.activation(s2[:], delta[:], ACTF.Sigmoid)
            mask2 = gpool.tile([P, NTT, E], F32, tag="mask2")
            nc.vector.tensor_tensor(mask2[:], lm[:],
                                    _bcast(m2[:], lm[:]), op=AL.is_ge)
            dmask = gpool.tile([P, NTT, E], F32, tag="dmask")
            nc.vector.tensor_tensor(dmask[:], mask2[:], mask1[:],
                                    op=AL.subtract)
            gsc = gpool.tile([P, NTT, E], F32, tag="gsc")
            nc.vector.tensor_tensor(gsc[:], dmask[:],
                                    _bcast(s2[:], dmask[:]), op=AL.mult)
            g_bf = gpool.tile([P, NTT, E], BF16, tag="gbf")
            nc.vector.tensor_tensor(g_bf[:], gsc[:], mask1[:], op=AL.add)

            # ---- g2T/GB/HG interleaved with k3 tail; then L2 ----
            g2t_all = gpool.tile([E, TC], BF16, tag="g2t")
            hg = {}

            def g2t_block(b):
                g2_ps = lt_ps_pool.tile([E, TPB, P], BF16, tag="lt",
                                        name=f"g2T{b}")
                for bo in range(TPB):
                    t = b * TPB + bo
                    nc.tensor.transpose(g2_ps[:, bo, :], g_bf[:, t, :],
                                        identb)
                nc.vector.tensor_copy(g2t_all[:, blk_cols(b)], g2_ps[:])

            def gb_one(k, b):
                gb_ps = lt_ps_pool.tile([P, LBLK], F32, tag="lt",
                                        name=f"gb{k}_{b}")
                nc.tensor.matmul(gb_ps[:], eblk(k),
                                 g2t_all[:, blk_cols(b)],
                                 start=True, stop=True)
                hg[(k, b)] = wpool.tile([P, LBLK], BF16, tag=f"hg{k}_{b}",
                                        name=f"hg{k}_{b}")
                nc.vector.tensor_tensor(hg[(k, b)][:], rk[(k, b)][:],
                                        gb_ps[:], op=AL.mult)

            def l2_tile(t):
                b, bo = divmod(t, TPB)
                tok = bass.ts(bo, P)
                o0 = ht_ps_pool.tile([P, LBLK], F32, tag="ht", name=f"o0_{t}")
                o1 = ht_ps_pool.tile([P, LBLK], F32, tag="ht", name=f"o1_{t}")
                last = BC - 1
                for k in range(BC):
                    st = (k == 0)
                    sp = (k == last) and not include_bu
                    nc.tensor.matmul(o0[:], hg[(k, b)][:, tok],
                                     wu_sb[:, k, 0:LBLK], start=st, stop=sp)
                    nc.tensor.matmul(o1[:], hg[(k, b)][:, tok],
                                     wu_sb[:, k, LBLK:2 * LBLK],
                                     start=st, stop=sp)
                if include_bu:
                    nc.tensor.matmul(o0[:], g2t_all[:, bass.ts(t, P)],
                                     bu_sb[:, 0:LBLK], start=False, stop=True)
                    nc.tensor.matmul(o1[:], g2t_all[:, bass.ts(t, P)],
                                     bu_sb[:, LBLK:2 * LBLK],
                                     start=False, stop=True)
                o_sb = opool.tile([P, D], BF16, tag="osb")
                nc.scalar.copy(o_sb[:, 0:LBLK], o0[:])
                nc.scalar.dma_start(out_d[bass.ts(t, P), 0:LBLK],
                                    o_sb[:, 0:LBLK])
                nc.vector.tensor_copy(o_sb[:, LBLK:2 * LBLK], o1[:])
                nc.gpsimd.dma_start(out_d[bass.ts(t, P), LBLK:2 * LBLK],
                                    o_sb[:, LBLK:2 * LBLK])

            ht[(3, 0)] = ht_ps_pool.tile([P, LBLK], F32, tag="ht",
                                         name="ht3_0")
            ht[(3, 1)] = ht_ps_pool.tile([P, LBLK], F32, tag="ht",
                                         name="ht3_1")
            for c in range(6):
                for b in range(NLB):
                    nc.tensor.matmul(ht[(3, b)][:], wd(c, 3),
                                     xhb(c, b),
                                     start=(c == 0), stop=False)
            # gate-prep for block 0 slots between k3 chunk groups so the
            # hg chain is ready the moment L2 starts
            g2t_block(0)
            for k in range(3):
                gb_one(k, 0)
            for c in range(6, KC):
                for b in range(NLB):
                    nc.tensor.matmul(ht[(3, b)][:], wd(c, 3),
                                     xhb(c, b),
                                     start=False, stop=(c == KC - 1))
            for b in range(NLB):
                drain_relu(3, b)
            gb_one(3, 0)
            l2_tile(0)
            g2t_block(1)
            for k in range(BC):
                gb_one(k, 1)
            for t in (1, 2, 3, 4, 5, 6, 7):
                l2_tile(t)

    nc.compile()
    _BUILD_CACHE[key] = nc
    return nc


def _split_bf16(a):
    hi = a.astype(ml_dtypes.bfloat16)
    lo = (a - hi.astype(np.float32)).astype(ml_dtypes.bfloat16)
    return hi, lo


def kernel(x, w_gate, w_noise, Wd, bd, Wu, bu, reps: int = 1):
    x = np.ascontiguousarray(np.asarray(x, dtype=np.float32))
    assert x.shape == (B_DIM, S_DIM, D), x.shape
    wg = np.ascontiguousarray(np.asarray(w_gate, dtype=np.float32))
    Wd = np.asarray(Wd, dtype=np.float32)
    Wu = np.asarray(Wu, dtype=np.float32)
    bd = np.asarray(bd, dtype=np.float32)
    bu = np.asarray(bu, dtype=np.float32)

    include_bd = bool(np.any(bd))
    include_bu = bool(np.any(bu))
    nc = _build(include_bd, include_bu, reps)

    xf = x.reshape(T, D)
    xh, xl = _split_bf16(xf)
    xht_full = np.ascontiguousarray(xh.T)   # [D, T] bf16
    xlt_full = np.ascontiguousarray(xl.T)
    wgh, wgl = _split_bf16(wg)              # [D, E] bf16
    wga = np.stack([np.asarray(wgh).reshape(KC, P, E).transpose(1, 0, 2),
                    np.asarray(wgl).reshape(KC, P, E).transpose(1, 0, 2)],
                   axis=2)                   # [P, KC, 2, E]
    wd_all = np.ascontiguousarray(
        Wd.transpose(1, 0, 2).reshape(D, EB)).astype(ml_dtypes.bfloat16)
    wu_flat = np.ascontiguousarray(
        (0.5 * Wu).reshape(EB, D)).astype(ml_dtypes.bfloat16)

    wga_flat = np.ascontiguousarray(
        np.asarray(wga, dtype=ml_dtypes.bfloat16).reshape(P, P))
    cstb = np.zeros((P, CG, P), dtype=ml_dtypes.bfloat16)
    cstb[:, 0, :] = np.eye(P, dtype=np.float32)
    eblk = np.kron(np.eye(E, dtype=np.float32),
                   np.ones((1, BK), dtype=np.float32))  # [E, EB]
    for k in range(BC):
        cstb[0:E, 1 + k, :] = eblk[:, k * P:(k + 1) * P]
    identf = np.eye(P, dtype=np.float32)

    shared = dict(wu=wu_flat, wga=wga_flat, cstb=cstb, identf=identf)
    if include_bd:
        shared["bd"] = np.ascontiguousarray(
            bd.reshape(EB)[np.arange(P)[:, None] + P * np.arange(BC)[None]])
    if include_bu:
        shared["bu"] = np.ascontiguousarray(
            (0.5 * bu).astype(ml_dtypes.bfloat16))

    in_maps = []
    for c in range(N_CORES):
        sl = slice(c * TC, (c + 1) * TC)
        xw = np.concatenate([xht_full[:, sl], wd_all], axis=1)
        in_maps.append(dict(xw=np.ascontiguousarray(xw),
                            xl=np.ascontiguousarray(xlt_full[:, sl]),
                            **shared))
    kernel.last_in_maps = in_maps
    res = run_bass_kernel_spmd(nc, in_maps, core_ids=list(range(N_CORES)))
    out = np.concatenate([np.asarray(res.results[c]["out"])
                          .astype(np.float32) for c in range(N_CORES)], axis=0)
    return out.reshape(B_DIM, S_DIM, D)
